# revision 19
# baseline (speedup 1.0000x reference)
"""LSTM cell (4096x1024, H=1024) as a Bass/Tile kernel on 8 TRN2 NeuronCores.

Sharding: 2D grid — 4 batch-quarters x 2 H-halves. Core c = 2*i + j gets
batch rows [i*1024,(i+1)*1024) and gate-output columns [j*512,(j+1)*512).
Each core computes gates = x @ W_j + h_prev @ U_j + b_j for its H-half
(gate order cbar, i, f, o), then c = f*c_prev + i*cbar, h = o*tanh(c).
No collectives: the host scatters inputs and gathers the output shards.

Per-core dataflow (bf16 GEMMs, transposed-gates orientation):
  - The host pre-transposes and concatenates activations into
    xh[m] = [x_shard | h_shard].T  ([2048, 512] bf16 per batch half) and
    pre-concatenates weights into wu[g] = [W_g; U_g]  ([2048, 512] bf16),
    so the kernel does zero on-chip transposes and the GEMM inputs are
    bf16 (1 PE cycle/row vs fp32's 4).
  - Gates are computed transposed: psum[n, m] = sum_ka wu[g][ka, n-tile]^T
    @ xh[m][ka, :].  wu tiles are the stationary operand in natural DRAM
    layout; xh is the moving operand (512-wide streams).
  - The two batch-half accumulation chains are interleaved ka-by-ka so
    consecutive matmuls share their stationary tile; dedup_ldweights()
    then deletes the redundant second Ldweights (the PE array is stateful
    and single-buffered — verified on HW), halving weight-load traffic
    and doubling the per-load stream length to 1024 rows.
  - The per-gate bias is applied for free on the ACT engine's per-partition
    bias port (gate cols are the partition dim in this orientation), which
    also eliminates the bias-seed matmuls.
  - ACT applies sigmoid/tanh straight out of PSUM; DVE does the gating;
    outputs h^T/c^T are DMA'd out on the Activation engine's HWDGE ring
    (stores never queue ahead of the next body's loads on the SP ring)
    and un-transposed on the host.
  - xh loads are split into 4-ka chunks so the first chains start after
    ~1MB lands instead of 4MB; input pools hold two bodies' worth of
    buffers so back-to-back invocations pipeline without PE gaps.
  - The bias tile is double-buffered (bias_bufs=2).  With a single
    buffer, the next body's bias load — first on the SP HWDGE ring — is
    WAR-blocked until the previous body's LAST activation reads the old
    bias, and since engine queues are in-order FIFO, every load behind
    it (14MB of xh/wu/cT) convoys, collapsing the cross-body pipeline.
  - The c_prev load is issued LAST on the ring (ct_last=True): its slot
    (ctp bufs=1) is WAR-held until ~85% through the previous body, and
    anything queued behind it would stall; last place = nothing behind.
"""

import numpy as np
import ml_dtypes
from contextlib import ExitStack

import bass_rust
import concourse.bass as bass
import concourse.mybir as mybir
import concourse.tile as tile
from concourse.vector_clock import ScopedClock
from concourse.bass_utils import run_bass_kernel_spmd

f32 = mybir.dt.float32
bf16 = mybir.dt.bfloat16
AFT = mybir.ActivationFunctionType
P = 128

B, E, H = 4096, 1024, 1024
BB, BH = 4, 2              # batch quarters x H halves
BL = B // BB               # 1024 rows per core
HL = H // BH               # 512 gate cols per core
NG = 4                     # gate order: cbar, i, f, o
KA = E + H                 # 2048 contraction (x|h concatenated)
KT = KA // P               # 16 k-tiles
NT = HL // P               # 4 n-tiles per gate
MC = 2                     # batch-half chunks per core
MW = BL // MC              # 512 moving cols per chunk (one PSUM bank)

NPBF16 = ml_dtypes.bfloat16


class PatchedTC(tile.TileContext):
    # This neuronxcc's core_v3 CTRL (Drain/NoOp) struct carries no sync-wait
    # slots, so the Tile tail-drain's waits must ride on EVSEM instructions.
    def _drain_and_barrier(self, tick_clock, wait_clock):
        tmp = mybir.InstNoOp(name=f"I-{self.nc.next_id()}",
                             engine=mybir.EngineType.SP)
        wait_clock.add_sem_waits(tmp, ScopedClock({None: tick_clock.global_clock}))
        sync = tmp.sync_info
        if sync is not None:
            for w in sync.on_wait:
                sem = bass_rust.SemaphoreHandle(w.ant_name, w.id)
                self.nc.sync.wait_ge(sem, w.wait_value)
        self.nc.sync.drain()
        self.nc.all_engine_barrier()
        popped = self.nc._tile_sem_poison_stack.pop()
        assert popped is self._sem_poison
        self.nc.clear_and_free_semaphores(list(self.sems.allocated().values()))
        self.nc.all_engine_barrier()


_SPLIT_SEQ = [0]


def split_multiwaits(nc, default_max=1, opcode_max=None):
    """This walrus build accepts at most one sync wait per instruction (zero
    for CTRL-struct ops like Drain/NoOp). Move excess waits onto dedicated
    EventSemaphore instructions inserted just before, on the same engine —
    semantically identical on an in-order engine queue."""
    opcode_max = opcode_max or {"Drain": 0, "NoOp": 0}
    for fn in nc.m.functions:
        for blk in fn.blocks:
            cur = blk.instructions
            out, changed = [], False
            for ins in cur:
                si = ins.sync_info
                waits = list(si.on_wait) if si is not None and si.on_wait else []
                cap = opcode_max.get(ins.opcode, default_max)
                if len(waits) > cap:
                    keep = waits[len(waits) - cap:] if cap else []
                    spill = waits[:len(waits) - cap]
                    for w in spill:
                        _SPLIT_SEQ[0] += 1
                        ev = mybir.InstEventSemaphore(
                            name=f"I-evw{_SPLIT_SEQ[0]}", engine=ins.engine)
                        ev.sync_info = bass_rust.SyncInfo(
                            on_wait=[w], on_update=[])
                        out.append(ev)
                    ins.sync_info = bass_rust.SyncInfo(
                        on_wait=keep, on_update=list(si.on_update))
                    changed = True
                out.append(ins)
            if changed:
                blk.instructions = out
    return nc


def hoist_ldweights(nc):
    """Software weight-preload: within each run of [LW, MM, LW, MM, ...] on
    the PE queue, move every LW one matmul earlier (LW1 LW2 MM1 LW3 MM2 ...
    MMn).  Matmult order (and thus the PE clock semaphore) is unchanged;
    Ldweights carry no semaphore updates, so sync bookkeeping is preserved."""
    for fn in nc.m.functions:
        for blk in fn.blocks:
            insts = blk.instructions
            pe_pos = [i for i, ins in enumerate(insts)
                      if ins.engine == mybir.EngineType.PE]
            pe_seq = [insts[i] for i in pe_pos]
            # transform maximal LW,MM,LW,MM... runs
            new_seq, i, n = [], 0, len(pe_seq)
            while i < n:
                if (pe_seq[i].opcode == "Ldweights" and i + 1 < n
                        and pe_seq[i + 1].opcode == "Matmult"):
                    j = i
                    pairs = []
                    while (j + 1 < n and pe_seq[j].opcode == "Ldweights"
                           and pe_seq[j + 1].opcode == "Matmult"):
                        pairs.append((pe_seq[j], pe_seq[j + 1]))
                        j += 2
                    new_seq.append(pairs[0][0])
                    for k in range(1, len(pairs)):
                        new_seq.append(pairs[k][0])
                        new_seq.append(pairs[k - 1][1])
                    new_seq.append(pairs[-1][1])
                    i = j
                else:
                    new_seq.append(pe_seq[i])
                    i += 1
            assert len(new_seq) == len(pe_seq)
            out = list(insts)
            for pos, ins in zip(pe_pos, new_seq):
                out[pos] = ins
            blk.instructions = out
    return nc


def dedup_ldweights(nc):
    """The PE array is stateful and single-buffered (verified on HW): a
    Matmult uses whatever the last Ldweights loaded.  Delete an Ldweights
    whose weights AP is byte-identical to the previous one when only
    Matmults sit between them — the reload is a no-op that costs 128 PE
    cycles.  Any sync waits on a deleted LW move to the next instruction."""
    for fn in nc.m.functions:
        for blk in fn.blocks:
            out = []
            last_lw_key = None
            pending_waits = []
            for ins in blk.instructions:
                if ins.engine != mybir.EngineType.PE:
                    out.append(ins)
                    continue
                if ins.opcode == "Ldweights":
                    key = str(ins.ins[0])
                    if key == last_lw_key:
                        si = ins.sync_info
                        if si is not None and si.on_wait:
                            pending_waits.extend(si.on_wait)
                        continue
                    last_lw_key = key
                elif ins.opcode not in ("Matmult", "EventSemaphore",
                                        "RegisterMove"):
                    # Only instructions that can clobber PE array state (or
                    # transfer control) invalidate the loaded weights; pure
                    # sync/register ops between duplicate LWs are safe.
                    last_lw_key = None
                if pending_waits:
                    si = ins.sync_info
                    waits = list(si.on_wait) if si is not None and si.on_wait \
                        else []
                    upds = list(si.on_update) if si is not None and \
                        si.on_update else []
                    ins.sync_info = bass_rust.SyncInfo(
                        on_wait=pending_waits + waits, on_update=upds)
                    pending_waits = []
                out.append(ins)
            blk.instructions = out
    return nc


def build_nc(split=True, repeat=1, loads_once=False, dma_only=False,
             probe_mw=None, hoist=False, pairm=True, store_ring=True,
             chunk_loads=True, bias_bufs=2, use_bias=True, do_stores=True,
             wu_bufs=NG, ct_last=True, grid=(BB, BH)):
    BB_, BH_ = grid
    BL = B // BB_              # batch rows per core
    HL = H // BH_              # gate cols per core
    NT = HL // P               # n-tiles per gate
    MC = BL // MW              # moving chunks per core
    # grid (2,4) needs 2 bodies of xh (16MB); shrink persists to bf16 to fit
    persist_dt = f32 if grid == (4, 2) else bf16
    XCH = 4 if chunk_loads else 1          # ka-chunks per xh[m] load
    KC = KT // XCH
    nc = bass.Bass(target_bir_lowering=False)
    xh_d = nc.declare_dram_parameter("xh", [MC, KA, MW], bf16, isOutput=False)
    wu_d = nc.declare_dram_parameter("wu", [NG, KA, HL], bf16, isOutput=False)
    bT_d = nc.declare_dram_parameter("bT", [P, NG * NT], f32, isOutput=False)
    cT_d = nc.declare_dram_parameter("cT_prev", [HL, BL], f32, isOutput=False)
    hT_o = nc.declare_dram_parameter("hT_out", [HL, BL], f32, isOutput=True)
    cT_o = nc.declare_dram_parameter("cT_out", [HL, BL], f32, isOutput=True)

    with PatchedTC(nc) as tc:
        with ExitStack() as ctx:
            biasp = ctx.enter_context(tc.tile_pool(name="biasp",
                                                   bufs=bias_bufs))
            xhp = ctx.enter_context(
                tc.tile_pool(name="xhp",
                             bufs=(2 * MC if pairm else MC) * XCH))
            wup = ctx.enter_context(tc.tile_pool(name="wup", bufs=wu_bufs))
            ctp = ctx.enter_context(tc.tile_pool(name="ctp", bufs=1))
            persist = ctx.enter_context(
                tc.tile_pool(name="persist", bufs=1 if pairm else 2))
            work = ctx.enter_context(tc.tile_pool(name="work", bufs=3))
            gpsum = ctx.enter_context(
                tc.tile_pool(name="gpsum", bufs=8, space="PSUM"))

            state = {}

            def emit_loads():
                # DMA ring order = need order: bias + first batch-half + the
                # first two gate matrices up front; c_prev / second half / last
                # gates stream in behind the first GEMM phases.
                bias_sb = biasp.tile([P, NG * NT], f32, tag="bias")
                nc.sync.dma_start(bias_sb[:], bT_d[:, :])

                xh_sb = [[None] * XCH for _ in range(MC)]
                wu_sb = [None] * NG

                def load_xh(m, c):
                    t = xhp.tile([P, KC, MW], bf16, tag="xh",
                                 name=f"xh{m}_{c}")
                    src = xh_d[m].rearrange("(k p) m -> p k m", p=P)
                    nc.sync.dma_start(t[:], src[:, c * KC:(c + 1) * KC, :])
                    xh_sb[m][c] = t

                def load_wu(g):
                    t = wup.tile([P, KT, HL], bf16, tag="wu")
                    nc.sync.dma_start(
                        t[:], wu_d[g].rearrange("(k p) c -> p k c", p=P))
                    wu_sb[g] = t

                for m in range(MC if pairm else 1):
                    load_xh(m, 0)
                load_wu(0)
                for c in range(1, XCH):
                    for m in range(MC if pairm else 1):
                        load_xh(m, c)
                load_wu(1)

                def load_ct():
                    t = ctp.tile([P, NT, BL], f32, tag="ct", name="cT_sb")
                    nc.sync.dma_start(
                        t[:], cT_d.rearrange("(n p) m -> p n m", p=P))
                    return t

                cT_sb = None if ct_last else load_ct()
                if not pairm:
                    for m in range(1, MC):
                        for c in range(XCH):
                            load_xh(m, c)
                load_wu(2)
                load_wu(3)
                if ct_last:
                    cT_sb = load_ct()
                state.update(bias_sb=bias_sb, xh_sb=xh_sb, wu_sb=wu_sb,
                             cT_sb=cT_sb)

            def emit_body():
                if not loads_once or not state:
                    emit_loads()
                bias_sb = state["bias_sb"]
                xh_sb = state["xh_sb"]
                wu_sb = state["wu_sb"]
                cT_sb = state["cT_sb"]
                if probe_mw is not None:
                    # Timing probe: same 512-matmul structure, truncated
                    # moving width; outputs never stored (timing only).
                    pw = probe_mw
                    for m in range(MC):
                        for g in range(NG):
                            for n in range(NT):
                                ps = gpsum.tile([P, pw], f32, tag="gp")
                                for ka in range(KT):
                                    nc.tensor.matmul(
                                        ps[:],
                                        wu_sb[g][:, ka, n * P:(n + 1) * P],
                                        xh_sb[m][ka // KC][:, ka % KC, 0:pw],
                                        start=(ka == 0), stop=(ka == KT - 1))
                                sg = work.tile([P, pw], f32, tag="sig")
                                nc.scalar.activation(sg[:], ps[:], AFT.Sigmoid)
                    z = work.tile([P, MW], f32, tag="outst")
                    nc.gpsimd.memset(z[:], 0.0)
                    for m in range(MC):
                        ms = slice(m * MW, (m + 1) * MW)
                        for n in range(NT):
                            nc.sync.dma_start(
                                cT_o[n * P:(n + 1) * P, ms], z[:])
                            nc.sync.dma_start(
                                hT_o[n * P:(n + 1) * P, ms], z[:])
                    return
                if dma_only:
                    z = work.tile([P, MW], f32, tag="outst")
                    nc.vector.tensor_copy(z[:], cT_sb[:, 0, 0:MW])
                    for m in range(MC):
                        ms = slice(m * MW, (m + 1) * MW)
                        for n in range(NT):
                            nc.sync.dma_start(
                                cT_o[n * P:(n + 1) * P, ms], z[:])
                            nc.sync.dma_start(
                                hT_o[n * P:(n + 1) * P, ms], z[:])
                    return

                store = nc.scalar.dma_start if store_ring \
                    else nc.sync.dma_start
                cbar = persist.tile([P, NT, BL], persist_dt, tag="cbar")
                tnc = persist.tile([P, NT, BL], persist_dt, tag="tnc")

                def gate_tail(g, n, m, ps):
                    ms = slice(m * MW, (m + 1) * MW)
                    bias_ap = bias_sb[:, g * NT + n:g * NT + n + 1] \
                        if use_bias else 0.0
                    if g == 0:
                        nc.scalar.activation(
                            cbar[:, n, ms], ps[:], AFT.Tanh, bias=bias_ap)
                    elif g == 1:
                        sig = work.tile([P, MW], f32, tag="sig")
                        nc.scalar.activation(
                            sig[:], ps[:], AFT.Sigmoid, bias=bias_ap)
                        nc.vector.tensor_mul(
                            cbar[:, n, ms], sig[:], cbar[:, n, ms])
                    elif g == 2:
                        sig = work.tile([P, MW], f32, tag="sig")
                        nc.scalar.activation(
                            sig[:], ps[:], AFT.Sigmoid, bias=bias_ap)
                        ct = work.tile([P, MW], f32, tag="outst")
                        nc.vector.tensor_mul(ct[:], sig[:], cT_sb[:, n, ms])
                        nc.vector.tensor_add(ct[:], ct[:], cbar[:, n, ms])
                        if do_stores:
                            store(cT_o[n * P:(n + 1) * P, ms], ct[:])
                        nc.scalar.activation(tnc[:, n, ms], ct[:], AFT.Tanh)
                    else:
                        sig = work.tile([P, MW], f32, tag="sig")
                        nc.scalar.activation(
                            sig[:], ps[:], AFT.Sigmoid, bias=bias_ap)
                        ht = work.tile([P, MW], f32, tag="outst")
                        nc.vector.tensor_mul(ht[:], sig[:], tnc[:, n, ms])
                        if do_stores:
                            store(hT_o[n * P:(n + 1) * P, ms], ht[:])

                if pairm:
                    # Both batch-half chains interleaved ka-by-ka: consecutive
                    # matmuls share the stationary tile, so dedup_ldweights
                    # halves the Ldweights count (the PE array is stateful).
                    for g in range(NG):
                        for n in range(NT):
                            pss = [gpsum.tile([P, MW], f32, tag="gp",
                                              name=f"ps{g}_{n}_{m}")
                                   for m in range(MC)]
                            for ka in range(KT):
                                for m in range(MC):
                                    nc.tensor.matmul(
                                        pss[m][:],
                                        wu_sb[g][:, ka, n * P:(n + 1) * P],
                                        xh_sb[m][ka // KC][:, ka % KC, :],
                                        start=(ka == 0), stop=(ka == KT - 1))
                            for m in range(MC):
                                gate_tail(g, n, m, pss[m])
                else:
                    for m in range(MC):
                        for g in range(NG):
                            for n in range(NT):
                                ps = gpsum.tile([P, MW], f32, tag="gp")
                                for ka in range(KT):
                                    nc.tensor.matmul(
                                        ps[:],
                                        wu_sb[g][:, ka, n * P:(n + 1) * P],
                                        xh_sb[m][ka // KC][:, ka % KC, :],
                                        start=(ka == 0), stop=(ka == KT - 1))
                                gate_tail(g, n, m, ps)

            for _ in range(repeat):
                emit_body()
    if hoist:
        hoist_ldweights(nc)
    if pairm:
        dedup_ldweights(nc)
    return split_multiwaits(nc) if split else nc


_NC_CACHE = {}


def _get_nc(key=()):
    if key not in _NC_CACHE:
        _NC_CACHE[key] = build_nc()
    return _NC_CACHE[key]


def make_in_maps(x, h_prev, c_prev, W, U, b, grid=(BB, BH)):
    """W/U: [NG, E|H, H] stacked gate-major (cbar, i, f, o); b: [NG, H]."""
    BB_, BH_ = grid
    BL = B // BB_
    HL = H // BH_
    NT = HL // P
    MC = BL // MW
    in_maps = []
    for core in range(BB_ * BH_):
        i, j = divmod(core, BH_)
        rs = slice(i * BL, (i + 1) * BL)
        cs = slice(j * HL, (j + 1) * HL)
        xh = np.concatenate([x[rs].T, h_prev[rs].T], axis=0)       # [KA, BL]
        xh = np.ascontiguousarray(
            xh.reshape(KA, MC, MW).transpose(1, 0, 2)).astype(NPBF16)
        wu = np.concatenate([W[:, :, cs], U[:, :, cs]], axis=1)    # [NG,KA,HL]
        wu = np.ascontiguousarray(wu).astype(NPBF16)
        bT = np.ascontiguousarray(
            b[:, cs].reshape(NG, NT, P).transpose(2, 0, 1).reshape(P, NG * NT))
        cT = np.ascontiguousarray(c_prev[rs, cs].T)                # [HL, BL]
        in_maps.append({"xh": xh, "wu": wu, "bT": bT, "cT_prev": cT})
    return in_maps


def assemble(results, grid=(BB, BH)):
    """results[core] -> {"hT_out": [HL,BL], "cT_out": [HL,BL]} -> [2,B,H]."""
    BB_, BH_ = grid
    BL = B // BB_
    HL = H // BH_
    h = np.empty((B, H), np.float32)
    c = np.empty((B, H), np.float32)
    for core in range(BB_ * BH_):
        i, j = divmod(core, BH_)
        rs = slice(i * BL, (i + 1) * BL)
        cs = slice(j * HL, (j + 1) * HL)
        h[rs, cs] = results[core]["hT_out"].T
        c[rs, cs] = results[core]["cT_out"].T
    return np.stack([h, c])


def kernel(**inputs):
    x = np.asarray(inputs["x"], np.float32)
    hm = np.asarray(inputs["hidden_memory_tm1"], np.float32)
    h_prev, c_prev = hm[0], hm[1]
    W = np.stack([np.asarray(inputs[k], np.float32)
                  for k in ("Wc", "Wi", "Wf", "Wog")])
    U = np.stack([np.asarray(inputs[k], np.float32)
                  for k in ("Uc", "Ui", "Uf", "Uog")])
    b = np.stack([np.asarray(inputs[k], np.float32)
                  for k in ("bc", "bi", "bf", "bog")])

    nc = _get_nc()
    res = run_bass_kernel_spmd(nc, make_in_maps(x, h_prev, c_prev, W, U, b),
                               list(range(BB * BH)))
    return assemble(res.results)



# revision 20
# speedup vs baseline: 1.1084x; 1.1084x over previous
"""LSTM cell (4096x1024, H=1024) as a Bass/Tile kernel on 8 TRN2 NeuronCores.

Sharding: 2D grid — 4 batch-quarters x 2 H-halves. Core c = 2*i + j gets
batch rows [i*1024,(i+1)*1024) and gate-output columns [j*512,(j+1)*512).
Each core computes gates = x @ W_j + h_prev @ U_j + b_j for its H-half
(gate order cbar, i, f, o), then c = f*c_prev + i*cbar, h = o*tanh(c).
No collectives: the host scatters inputs and gathers the output shards.

Per-core dataflow (bf16 GEMMs, transposed-gates orientation):
  - The host pre-transposes and concatenates activations into
    xh[m] = [x_shard | h_shard].T  ([2048, 512] bf16 per batch half) and
    pre-concatenates weights into wu[g] = [W_g; U_g]  ([2048, 512] bf16),
    so the kernel does zero on-chip transposes and the GEMM inputs are
    bf16 (1 PE cycle/row vs fp32's 4).
  - Gates are computed transposed: psum[n, m] = sum_ka wu[g][ka, n-tile]^T
    @ xh[m][ka, :].  wu tiles are the stationary operand in natural DRAM
    layout; xh is the moving operand (512-wide streams).
  - The two batch-half accumulation chains are interleaved ka-by-ka so
    consecutive matmuls share their stationary tile; dedup_ldweights()
    then deletes the redundant second Ldweights (the PE array is stateful
    and single-buffered — verified on HW), halving weight-load traffic
    and doubling the per-load stream length to 1024 rows.
  - The per-gate bias is applied for free on the ACT engine's per-partition
    bias port (gate cols are the partition dim in this orientation), which
    also eliminates the bias-seed matmuls.
  - ACT applies sigmoid/tanh straight out of PSUM; DVE does the gating;
    outputs h^T/c^T are DMA'd out on the Activation engine's HWDGE ring
    (stores never queue ahead of the next body's loads on the SP ring)
    and un-transposed on the host.
  - xh loads are split into 4-ka chunks so the first chains start after
    ~1MB lands instead of 4MB; input pools hold two bodies' worth of
    buffers so back-to-back invocations pipeline without PE gaps.
  - The bias tile is double-buffered (bias_bufs=2).  With a single
    buffer, the next body's bias load — first on the SP HWDGE ring — is
    WAR-blocked until the previous body's LAST activation reads the old
    bias, and since engine queues are in-order FIFO, every load behind
    it (14MB of xh/wu/cT) convoys, collapsing the cross-body pipeline.
  - The c_prev load is issued LAST on the ring (ct_last=True): its slot
    (ctp bufs=1) is WAR-held until ~85% through the previous body, and
    anything queued behind it would stall; last place = nothing behind.
"""

import numpy as np
import ml_dtypes
from contextlib import ExitStack

import bass_rust
import concourse.bass as bass
import concourse.mybir as mybir
import concourse.tile as tile
from concourse.vector_clock import ScopedClock
from concourse.bass_utils import run_bass_kernel_spmd

f32 = mybir.dt.float32
bf16 = mybir.dt.bfloat16
AFT = mybir.ActivationFunctionType
P = 128

B, E, H = 4096, 1024, 1024
BB, BH = 4, 2              # batch quarters x H halves
BL = B // BB               # 1024 rows per core
HL = H // BH               # 512 gate cols per core
NG = 4                     # gate order: cbar, i, f, o
KA = E + H                 # 2048 contraction (x|h concatenated)
KT = KA // P               # 16 k-tiles
NT = HL // P               # 4 n-tiles per gate
MC = 2                     # batch-half chunks per core
MW = BL // MC              # 512 moving cols per chunk (one PSUM bank)

NPBF16 = ml_dtypes.bfloat16


class PatchedTC(tile.TileContext):
    # This neuronxcc's core_v3 CTRL (Drain/NoOp) struct carries no sync-wait
    # slots, so the Tile tail-drain's waits must ride on EVSEM instructions.
    def _drain_and_barrier(self, tick_clock, wait_clock):
        tmp = mybir.InstNoOp(name=f"I-{self.nc.next_id()}",
                             engine=mybir.EngineType.SP)
        wait_clock.add_sem_waits(tmp, ScopedClock({None: tick_clock.global_clock}))
        sync = tmp.sync_info
        if sync is not None:
            for w in sync.on_wait:
                sem = bass_rust.SemaphoreHandle(w.ant_name, w.id)
                self.nc.sync.wait_ge(sem, w.wait_value)
        self.nc.sync.drain()
        self.nc.all_engine_barrier()
        popped = self.nc._tile_sem_poison_stack.pop()
        assert popped is self._sem_poison
        self.nc.clear_and_free_semaphores(list(self.sems.allocated().values()))
        self.nc.all_engine_barrier()


_SPLIT_SEQ = [0]


def split_multiwaits(nc, default_max=1, opcode_max=None):
    """This walrus build accepts at most one sync wait per instruction (zero
    for CTRL-struct ops like Drain/NoOp). Move excess waits onto dedicated
    EventSemaphore instructions inserted just before, on the same engine —
    semantically identical on an in-order engine queue."""
    opcode_max = opcode_max or {"Drain": 0, "NoOp": 0}
    for fn in nc.m.functions:
        for blk in fn.blocks:
            cur = blk.instructions
            out, changed = [], False
            for ins in cur:
                si = ins.sync_info
                waits = list(si.on_wait) if si is not None and si.on_wait else []
                cap = opcode_max.get(ins.opcode, default_max)
                if len(waits) > cap:
                    keep = waits[len(waits) - cap:] if cap else []
                    spill = waits[:len(waits) - cap]
                    for w in spill:
                        _SPLIT_SEQ[0] += 1
                        ev = mybir.InstEventSemaphore(
                            name=f"I-evw{_SPLIT_SEQ[0]}", engine=ins.engine)
                        ev.sync_info = bass_rust.SyncInfo(
                            on_wait=[w], on_update=[])
                        out.append(ev)
                    ins.sync_info = bass_rust.SyncInfo(
                        on_wait=keep, on_update=list(si.on_update))
                    changed = True
                out.append(ins)
            if changed:
                blk.instructions = out
    return nc


def hoist_ldweights(nc):
    """Software weight-preload: within each run of [LW, MM, LW, MM, ...] on
    the PE queue, move every LW one matmul earlier (LW1 LW2 MM1 LW3 MM2 ...
    MMn).  Matmult order (and thus the PE clock semaphore) is unchanged;
    Ldweights carry no semaphore updates, so sync bookkeeping is preserved."""
    for fn in nc.m.functions:
        for blk in fn.blocks:
            insts = blk.instructions
            pe_pos = [i for i, ins in enumerate(insts)
                      if ins.engine == mybir.EngineType.PE]
            pe_seq = [insts[i] for i in pe_pos]
            # transform maximal LW,MM,LW,MM... runs
            new_seq, i, n = [], 0, len(pe_seq)
            while i < n:
                if (pe_seq[i].opcode == "Ldweights" and i + 1 < n
                        and pe_seq[i + 1].opcode == "Matmult"):
                    j = i
                    pairs = []
                    while (j + 1 < n and pe_seq[j].opcode == "Ldweights"
                           and pe_seq[j + 1].opcode == "Matmult"):
                        pairs.append((pe_seq[j], pe_seq[j + 1]))
                        j += 2
                    new_seq.append(pairs[0][0])
                    for k in range(1, len(pairs)):
                        new_seq.append(pairs[k][0])
                        new_seq.append(pairs[k - 1][1])
                    new_seq.append(pairs[-1][1])
                    i = j
                else:
                    new_seq.append(pe_seq[i])
                    i += 1
            assert len(new_seq) == len(pe_seq)
            out = list(insts)
            for pos, ins in zip(pe_pos, new_seq):
                out[pos] = ins
            blk.instructions = out
    return nc


def dedup_ldweights(nc):
    """The PE array is stateful and single-buffered (verified on HW): a
    Matmult uses whatever the last Ldweights loaded.  Delete an Ldweights
    whose weights AP is byte-identical to the previous one when only
    Matmults sit between them — the reload is a no-op that costs 128 PE
    cycles.  Any sync waits on a deleted LW move to the next instruction."""
    for fn in nc.m.functions:
        for blk in fn.blocks:
            out = []
            last_lw_key = None
            pending_waits = []
            for ins in blk.instructions:
                if ins.engine != mybir.EngineType.PE:
                    out.append(ins)
                    continue
                if ins.opcode == "Ldweights":
                    key = str(ins.ins[0])
                    if key == last_lw_key:
                        si = ins.sync_info
                        if si is not None and si.on_wait:
                            pending_waits.extend(si.on_wait)
                        continue
                    last_lw_key = key
                elif ins.opcode not in ("Matmult", "EventSemaphore",
                                        "RegisterMove"):
                    # Only instructions that can clobber PE array state (or
                    # transfer control) invalidate the loaded weights; pure
                    # sync/register ops between duplicate LWs are safe.
                    last_lw_key = None
                if pending_waits:
                    si = ins.sync_info
                    waits = list(si.on_wait) if si is not None and si.on_wait \
                        else []
                    upds = list(si.on_update) if si is not None and \
                        si.on_update else []
                    ins.sync_info = bass_rust.SyncInfo(
                        on_wait=pending_waits + waits, on_update=upds)
                    pending_waits = []
                out.append(ins)
            blk.instructions = out
    return nc


def build_nc(split=True, repeat=1, loads_once=False, dma_only=False,
             probe_mw=None, hoist=False, pairm=True, store_ring=True,
             chunk_loads=True, bias_bufs=2, use_bias=True, do_stores=True,
             wu_bufs=NG, ct_last=True, grid=(BB, BH)):
    BB_, BH_ = grid
    BL = B // BB_              # batch rows per core
    HL = H // BH_              # gate cols per core
    NT = HL // P               # n-tiles per gate
    MC = BL // MW              # moving chunks per core
    # grid (2,4) needs 2 bodies of xh (16MB); shrink persists to bf16 to fit
    persist_dt = f32 if grid == (4, 2) else bf16
    XCH = 4 if chunk_loads else 1          # ka-chunks per xh[m] load
    KC = KT // XCH
    nc = bass.Bass(target_bir_lowering=False)
    xh_d = nc.declare_dram_parameter("xh", [MC, KA, MW], bf16, isOutput=False)
    wu_d = nc.declare_dram_parameter("wu", [NG, KA, HL], bf16, isOutput=False)
    bT_d = nc.declare_dram_parameter("bT", [P, NG * NT], f32, isOutput=False)
    cT_d = nc.declare_dram_parameter("cT_prev", [HL, BL], f32, isOutput=False)
    hT_o = nc.declare_dram_parameter("hT_out", [HL, BL], f32, isOutput=True)
    cT_o = nc.declare_dram_parameter("cT_out", [HL, BL], f32, isOutput=True)

    with PatchedTC(nc) as tc:
        with ExitStack() as ctx:
            biasp = ctx.enter_context(tc.tile_pool(name="biasp",
                                                   bufs=bias_bufs))
            xhp = ctx.enter_context(
                tc.tile_pool(name="xhp",
                             bufs=(2 * MC if pairm else MC) * XCH))
            wup = ctx.enter_context(tc.tile_pool(name="wup", bufs=wu_bufs))
            ctp = ctx.enter_context(tc.tile_pool(name="ctp", bufs=1))
            persist = ctx.enter_context(
                tc.tile_pool(name="persist", bufs=1 if pairm else 2))
            work = ctx.enter_context(tc.tile_pool(name="work", bufs=3))
            gpsum = ctx.enter_context(
                tc.tile_pool(name="gpsum", bufs=8, space="PSUM"))

            state = {}

            def emit_loads():
                # DMA ring order = need order: bias + first batch-half + the
                # first two gate matrices up front; c_prev / second half / last
                # gates stream in behind the first GEMM phases.
                bias_sb = biasp.tile([P, NG * NT], f32, tag="bias")
                nc.sync.dma_start(bias_sb[:], bT_d[:, :])

                xh_sb = [[None] * XCH for _ in range(MC)]
                wu_sb = [None] * NG

                def load_xh(m, c):
                    t = xhp.tile([P, KC, MW], bf16, tag="xh",
                                 name=f"xh{m}_{c}")
                    src = xh_d[m].rearrange("(k p) m -> p k m", p=P)
                    nc.sync.dma_start(t[:], src[:, c * KC:(c + 1) * KC, :])
                    xh_sb[m][c] = t

                def load_wu(g):
                    t = wup.tile([P, KT, HL], bf16, tag="wu")
                    nc.sync.dma_start(
                        t[:], wu_d[g].rearrange("(k p) c -> p k c", p=P))
                    wu_sb[g] = t

                for m in range(MC if pairm else 1):
                    load_xh(m, 0)
                load_wu(0)
                for c in range(1, XCH):
                    for m in range(MC if pairm else 1):
                        load_xh(m, c)
                load_wu(1)

                def load_ct():
                    t = ctp.tile([P, NT, BL], f32, tag="ct", name="cT_sb")
                    nc.sync.dma_start(
                        t[:], cT_d.rearrange("(n p) m -> p n m", p=P))
                    return t

                cT_sb = None if ct_last else load_ct()
                if not pairm:
                    for m in range(1, MC):
                        for c in range(XCH):
                            load_xh(m, c)
                load_wu(2)
                load_wu(3)
                if ct_last:
                    cT_sb = load_ct()
                state.update(bias_sb=bias_sb, xh_sb=xh_sb, wu_sb=wu_sb,
                             cT_sb=cT_sb)

            def emit_body():
                if not loads_once or not state:
                    emit_loads()
                bias_sb = state["bias_sb"]
                xh_sb = state["xh_sb"]
                wu_sb = state["wu_sb"]
                cT_sb = state["cT_sb"]
                if probe_mw is not None:
                    # Timing probe: same 512-matmul structure, truncated
                    # moving width; outputs never stored (timing only).
                    pw = probe_mw
                    for m in range(MC):
                        for g in range(NG):
                            for n in range(NT):
                                ps = gpsum.tile([P, pw], f32, tag="gp")
                                for ka in range(KT):
                                    nc.tensor.matmul(
                                        ps[:],
                                        wu_sb[g][:, ka, n * P:(n + 1) * P],
                                        xh_sb[m][ka // KC][:, ka % KC, 0:pw],
                                        start=(ka == 0), stop=(ka == KT - 1))
                                sg = work.tile([P, pw], f32, tag="sig")
                                nc.scalar.activation(sg[:], ps[:], AFT.Sigmoid)
                    z = work.tile([P, MW], f32, tag="outst")
                    nc.gpsimd.memset(z[:], 0.0)
                    for m in range(MC):
                        ms = slice(m * MW, (m + 1) * MW)
                        for n in range(NT):
                            nc.sync.dma_start(
                                cT_o[n * P:(n + 1) * P, ms], z[:])
                            nc.sync.dma_start(
                                hT_o[n * P:(n + 1) * P, ms], z[:])
                    return
                if dma_only:
                    z = work.tile([P, MW], f32, tag="outst")
                    nc.vector.tensor_copy(z[:], cT_sb[:, 0, 0:MW])
                    for m in range(MC):
                        ms = slice(m * MW, (m + 1) * MW)
                        for n in range(NT):
                            nc.sync.dma_start(
                                cT_o[n * P:(n + 1) * P, ms], z[:])
                            nc.sync.dma_start(
                                hT_o[n * P:(n + 1) * P, ms], z[:])
                    return

                store = nc.scalar.dma_start if store_ring \
                    else nc.sync.dma_start
                cbar = persist.tile([P, NT, BL], persist_dt, tag="cbar")
                tnc = persist.tile([P, NT, BL], persist_dt, tag="tnc")

                def gate_tail(g, n, m, ps):
                    ms = slice(m * MW, (m + 1) * MW)
                    bias_ap = bias_sb[:, g * NT + n:g * NT + n + 1] \
                        if use_bias else 0.0
                    if g == 0:
                        nc.scalar.activation(
                            cbar[:, n, ms], ps[:], AFT.Tanh, bias=bias_ap)
                    elif g == 1:
                        sig = work.tile([P, MW], f32, tag="sig")
                        nc.scalar.activation(
                            sig[:], ps[:], AFT.Sigmoid, bias=bias_ap)
                        nc.vector.tensor_mul(
                            cbar[:, n, ms], sig[:], cbar[:, n, ms])
                    elif g == 2:
                        sig = work.tile([P, MW], f32, tag="sig")
                        nc.scalar.activation(
                            sig[:], ps[:], AFT.Sigmoid, bias=bias_ap)
                        ct = work.tile([P, MW], f32, tag="outst")
                        nc.vector.tensor_mul(ct[:], sig[:], cT_sb[:, n, ms])
                        nc.vector.tensor_add(ct[:], ct[:], cbar[:, n, ms])
                        if do_stores:
                            store(cT_o[n * P:(n + 1) * P, ms], ct[:])
                        nc.scalar.activation(tnc[:, n, ms], ct[:], AFT.Tanh)
                    else:
                        sig = work.tile([P, MW], f32, tag="sig")
                        nc.scalar.activation(
                            sig[:], ps[:], AFT.Sigmoid, bias=bias_ap)
                        ht = work.tile([P, MW], f32, tag="outst")
                        nc.vector.tensor_mul(ht[:], sig[:], tnc[:, n, ms])
                        if do_stores:
                            store(hT_o[n * P:(n + 1) * P, ms], ht[:])

                if pairm:
                    # Both batch-half chains interleaved ka-by-ka: consecutive
                    # matmuls share the stationary tile, so dedup_ldweights
                    # halves the Ldweights count (the PE array is stateful).
                    for g in range(NG):
                        for n in range(NT):
                            pss = [gpsum.tile([P, MW], f32, tag="gp",
                                              name=f"ps{g}_{n}_{m}")
                                   for m in range(MC)]
                            for ka in range(KT):
                                for m in range(MC):
                                    nc.tensor.matmul(
                                        pss[m][:],
                                        wu_sb[g][:, ka, n * P:(n + 1) * P],
                                        xh_sb[m][ka // KC][:, ka % KC, :],
                                        start=(ka == 0), stop=(ka == KT - 1))
                            for m in range(MC):
                                gate_tail(g, n, m, pss[m])
                else:
                    for m in range(MC):
                        for g in range(NG):
                            for n in range(NT):
                                ps = gpsum.tile([P, MW], f32, tag="gp")
                                for ka in range(KT):
                                    nc.tensor.matmul(
                                        ps[:],
                                        wu_sb[g][:, ka, n * P:(n + 1) * P],
                                        xh_sb[m][ka // KC][:, ka % KC, :],
                                        start=(ka == 0), stop=(ka == KT - 1))
                                gate_tail(g, n, m, ps)

            for _ in range(repeat):
                emit_body()
    if hoist:
        hoist_ldweights(nc)
    if pairm:
        dedup_ldweights(nc)
    return split_multiwaits(nc) if split else nc


_NC_CACHE = {}


def _get_nc(key=()):
    if key not in _NC_CACHE:
        _NC_CACHE[key] = build_nc()
    return _NC_CACHE[key]


def make_in_maps(x, h_prev, c_prev, W, U, b, grid=(BB, BH)):
    """W/U: [NG, E|H, H] stacked gate-major (cbar, i, f, o); b: [NG, H]."""
    BB_, BH_ = grid
    BL = B // BB_
    HL = H // BH_
    NT = HL // P
    MC = BL // MW
    in_maps = []
    for core in range(BB_ * BH_):
        i, j = divmod(core, BH_)
        rs = slice(i * BL, (i + 1) * BL)
        cs = slice(j * HL, (j + 1) * HL)
        xh = np.concatenate([x[rs].T, h_prev[rs].T], axis=0)       # [KA, BL]
        xh = np.ascontiguousarray(
            xh.reshape(KA, MC, MW).transpose(1, 0, 2)).astype(NPBF16)
        wu = np.concatenate([W[:, :, cs], U[:, :, cs]], axis=1)    # [NG,KA,HL]
        wu = np.ascontiguousarray(wu).astype(NPBF16)
        bT = np.ascontiguousarray(
            b[:, cs].reshape(NG, NT, P).transpose(2, 0, 1).reshape(P, NG * NT))
        cT = np.ascontiguousarray(c_prev[rs, cs].T)                # [HL, BL]
        in_maps.append({"xh": xh, "wu": wu, "bT": bT, "cT_prev": cT})
    return in_maps


def assemble(results, grid=(BB, BH)):
    """results[core] -> {"hT_out": [HL,BL], "cT_out": [HL,BL]} -> [2,B,H]."""
    BB_, BH_ = grid
    BL = B // BB_
    HL = H // BH_
    h = np.empty((B, H), np.float32)
    c = np.empty((B, H), np.float32)
    for core in range(BB_ * BH_):
        i, j = divmod(core, BH_)
        rs = slice(i * BL, (i + 1) * BL)
        cs = slice(j * HL, (j + 1) * HL)
        h[rs, cs] = results[core]["hT_out"].T
        c[rs, cs] = results[core]["cT_out"].T
    return np.stack([h, c])


def kernel(**inputs):
    x = np.asarray(inputs["x"], np.float32)
    hm = np.asarray(inputs["hidden_memory_tm1"], np.float32)
    h_prev, c_prev = hm[0], hm[1]
    W = np.stack([np.asarray(inputs[k], np.float32)
                  for k in ("Wc", "Wi", "Wf", "Wog")])
    U = np.stack([np.asarray(inputs[k], np.float32)
                  for k in ("Uc", "Ui", "Uf", "Uog")])
    b = np.stack([np.asarray(inputs[k], np.float32)
                  for k in ("bc", "bi", "bf", "bog")])

    nc = _get_nc()
    in_maps = make_in_maps(x, h_prev, c_prev, W, U, b)
    # The axon/NRT stack very occasionally returns garbage from a wedged
    # device (observed once: values ~1e9 alongside NRT_EXEC_UNIT_
    # UNRECOVERABLE events; deterministic-correct on every re-run).  h is
    # tanh-bounded and |c| <~ tens, so an insane magnitude means a broken
    # execution, not a numerics issue — retry once before giving up.
    for attempt in range(2):
        res = run_bass_kernel_spmd(nc, in_maps, list(range(BB * BH)))
        out = assemble(res.results)
        if np.isfinite(out).all() and np.abs(out).max() < 1e4:
            return out
    return out

